# revision 9
# baseline (speedup 1.0000x reference)
"""AdaptiveGraphAttention Trainium2 kernel (8 NeuronCores, data-parallel).

Math: in the reference, logits[b,h,i,j] = a_q[b,h,i] + a_k[b,h,j] +
e_j[b,h,j]*adj[i,j] + attn_b with adj[:,0]=0, adj[:,1:]=1 — the mask and the
j-dependent terms are identical for every query row i, and the a_q/bias terms
are constant over j.  Softmax is shift-invariant, so the attention
distribution p[b,h,:] = softmax_{j>=1}(a_k + e_j) is the same for all i: the
attention matrix is rank-1 and the output is one row per batch, broadcast
over the 256 query positions.  bq/bk/attn_b cancel exactly; bv survives as
an additive constant (sum_j p_j = 1); bo is added on the host.

Per-head dots fold into small matrices:
  a_k[b,j,h] = nv[b,j,:] @ Uk[:,h],  Uk[d,h] = sum_m Wk[h*64+m, d] * w_k[m]
  e_j[b,j,h] = desc[b,j-1,:] @ Ue[:,h], Ue[h*64+m, h] = w_e[m] (else 0)

Device work per core (4 batches), bf16 matmul inputs / f32 accumulation:
  c[h,j]    = Uk.T @ nvT[:, j] + Ue.T @ descT[:, j-1]      (PE, PSUM accum)
  p[h,:]    = softmax_j(c)                                  (DVE/ACT)
  nvbarT    = nv.T @ p.T          [1024, 16] per batch      (PE)
  VbarT     = WvT.T @ nvbarT_all  [1024, 4*16]              (PE)
  ctxT      = blockdiag-select(VbarT) + bv   [1024, 4]      (DVE)
  out       = ctxT.T @ WoT        [4, 1024]                 (PE) -> DMA

All DRAM inputs are host-prepermuted to [128, chunk, inner] so each DMA
partition row is one contiguous run (descriptor-count relief).
"""

import numpy as np
import ml_dtypes
from contextlib import ExitStack

import concourse.bass as bass
import concourse.mybir as mybir
import concourse.tile as tile
from concourse import bacc
from concourse.bass_utils import run_bass_kernel_spmd
from concourse.masks import make_identity

B, S, D, H, HD = 32, 256, 1024, 16, 64
NCORES = 8
BPC = B // NCORES  # 4 batches per core
F32 = mybir.dt.float32
BF16 = mybir.dt.bfloat16
NPBF = ml_dtypes.bfloat16
F8 = mybir.dt.float8e4
NPF8 = ml_dtypes.float8_e4m3
USCALE = 512.0  # fp8 range lift for the tiny folded U entries
DC = D // 128  # 8 chunks of the model dim
JC = S // 128  # 2 chunks of the sequence dim

_cache = {}


def _build():
    nc = bacc.Bacc("TRN2", target_bir_lowering=False, debug=False,
                   num_devices=NCORES)

    nv_ext = nc.declare_dram_parameter("nv", [BPC, 128, JC, D], BF16,
                                       isOutput=False)
    nvt_ext = nc.declare_dram_parameter("nvT", [BPC, 128, DC, S], F8,
                                        isOutput=False)
    dst_ext = nc.declare_dram_parameter("descT", [BPC, 128, DC, S - 1], F8,
                                        isOutput=False)
    u_ext = nc.declare_dram_parameter("U", [128, DC, 2 * H], F8,
                                      isOutput=False)
    wvt_ext = nc.declare_dram_parameter("WvT", [128, DC, DC, 128], BF16,
                                        isOutput=False)
    wot_ext = nc.declare_dram_parameter("WoT", [128, DC, D], BF16,
                                        isOutput=False)
    bv_ext = nc.declare_dram_parameter("bv", [128, DC], F32, isOutput=False)
    out_ext = nc.declare_dram_parameter("out", [BPC, D], F32, isOutput=True)

    with tile.TileContext(nc) as tc, ExitStack() as ctx:
        wpool = ctx.enter_context(tc.tile_pool(name="w", bufs=1))
        xpool = ctx.enter_context(tc.tile_pool(name="x", bufs=2))
        smpool = ctx.enter_context(tc.tile_pool(name="sm", bufs=2))
        ps_s = ctx.enter_context(tc.tile_pool(name="ps_s", bufs=2, space="PSUM"))
        ps_pt = ctx.enter_context(tc.tile_pool(name="ps_pt", bufs=1, space="PSUM"))
        ps_nb = ctx.enter_context(tc.tile_pool(name="ps_nb", bufs=2, space="PSUM"))
        ps_vb = ctx.enter_context(tc.tile_pool(name="ps_vb", bufs=1, space="PSUM"))
        ps_o = ctx.enter_context(tc.tile_pool(name="ps_o", bufs=1, space="PSUM"))

        # --- resident constants (weights are DMA'd after the batch loop so
        # the batch-0 activation loads win the DMA queues) ------------------
        ident = wpool.tile([128, 128], F32)
        make_identity(nc, ident[:])
        u_sb = wpool.tile([128, DC, 2 * H], F8)
        nc.sync.dma_start(out=u_sb[:], in_=u_ext.ap())

        # nvbarT for all batches: [p, d-chunk, b*16+h]
        nvall = wpool.tile([128, DC, BPC * H], BF16)

        # weight tiles; chunk DMAs are interleaved between batch loads so
        # they fill queue slack without delaying activations
        wvt_sb = wpool.tile([128, DC, DC, 128], BF16)
        wot_sb = wpool.tile([128, DC, D], BF16)

        for b in range(BPC):
            # --- load this batch's activations ----------------------------
            nvt_sb = xpool.tile([128, DC, S], F8)
            nc.sync.dma_start(out=nvt_sb[:], in_=nvt_ext[b])
            dst_sb = xpool.tile([128, DC, S - 1], F8)
            nc.sync.dma_start(out=dst_sb[:], in_=dst_ext[b])
            nv_sb = xpool.tile([128, JC, D], BF16)
            nc.sync.dma_start(out=nv_sb[:], in_=nv_ext[b])
            for wc in range(2 * b, 2 * b + 2):
                nc.sync.dma_start(out=wvt_sb[:, wc], in_=wvt_ext[:, wc])
                nc.sync.dma_start(out=wot_sb[:, wc, :], in_=wot_ext[:, wc, :])

            # --- logits c[h, j] for j=1..255, accumulated in PSUM ---------
            psc = ps_s.tile([H, S - 1], F32)
            for c in range(DC):
                nc.tensor.matmul(psc[:], u_sb[:, c, 0:H], nvt_sb[:, c, 1:S],
                                 start=(c == 0), stop=False)
            for c in range(DC):
                nc.tensor.matmul(psc[:], u_sb[:, c, H:2 * H], dst_sb[:, c, :],
                                 start=False, stop=(c == DC - 1))

            # --- softmax over j (free dim) --------------------------------
            negmax = smpool.tile([H, 1], F32)
            nc.vector.reduce_max(out=negmax[:], in_=psc[:],
                                 axis=mybir.AxisListType.X, negate=True)
            negmax_s = smpool.tile([H, 1], F32)
            nc.vector.tensor_scalar_mul(negmax_s[:], negmax[:], 1.0 / USCALE)
            p_sb = smpool.tile([H, S], F32)
            nc.vector.memset(p_sb[:, 0:1], 0.0)
            sumx = smpool.tile([H, 1], F32)
            nc.scalar.activation(p_sb[:, 1:S], psc[:],
                                 mybir.ActivationFunctionType.Exp,
                                 bias=negmax_s[:], scale=1.0 / USCALE,
                                 accum_out=sumx[:])
            recip = smpool.tile([H, 1], F32)
            nc.vector.reciprocal(recip[:], sumx[:])
            nc.vector.tensor_scalar_mul(p_sb[:, 1:S], p_sb[:, 1:S], recip[:])

            # --- pT[j, h] via PE transpose --------------------------------
            pt_sb = smpool.tile([128, JC, H], BF16)
            for jc in range(JC):
                pt_ps = ps_pt.tile([128, H], F32)
                nc.tensor.transpose(pt_ps[:], p_sb[:, jc * 128:(jc + 1) * 128],
                                    ident[0:H, 0:H])
                nc.vector.tensor_copy(pt_sb[:, jc, :], pt_ps[:])

            # --- nvbarT[d, h] = sum_j nv[j, d] * p[j, h] ------------------
            for cm in range(DC):
                nb_ps = ps_nb.tile([128, H], F32)
                for jc in range(JC):
                    nc.tensor.matmul(nb_ps[:],
                                     nv_sb[:, jc, cm * 128:(cm + 1) * 128],
                                     pt_sb[:, jc, :],
                                     start=(jc == 0), stop=(jc == JC - 1))
                nc.vector.tensor_copy(nvall[:, cm, b * H:(b + 1) * H], nb_ps[:])

        bv_sb = wpool.tile([128, DC], F32)
        nc.sync.dma_start(out=bv_sb[:], in_=bv_ext.ap())

        # --- fused epilogue, pipelined per d'-chunk cm:
        #   VbarT[cm] = sum_ck WvT[ck, cm].T @ nvbarT   [128, 64]
        #   ctxT[cm]  = blockdiag-select + bv           [128, 4]
        #   out      += ctxT[cm].T @ WoT[cm]            [4, 1024] accum
        ctx_sb = wpool.tile([128, DC, BPC], BF16)
        o_ps = ps_o.tile([BPC, D], F32)
        for cm in range(DC):
            vb_ps = ps_vb.tile([128, BPC * H], F32)
            for ck in range(DC):
                nc.tensor.matmul(vb_ps[:],
                                 wvt_sb[:, cm, ck, :],
                                 nvall[:, ck, :],
                                 start=(ck == 0), stop=(ck == DC - 1))
            # block-diagonal select + bv: rows [64*half] take head 2*cm+half
            for half in range(2):
                h = 2 * cm + half
                rows = slice(64 * half, 64 * half + 64)
                src = vb_ps[rows, :].rearrange("p (b h) -> p b h", h=H)[:, :, h]
                nc.vector.tensor_scalar_add(ctx_sb[rows, cm, :], src,
                                            bv_sb[rows, cm:cm + 1])
            for n2 in range(2):
                cols = slice(n2 * 512, (n2 + 1) * 512)
                nc.tensor.matmul(o_ps[:, cols], ctx_sb[:, cm, :],
                                 wot_sb[:, cm, cols],
                                 start=(cm == 0), stop=(cm == DC - 1))
        o_sb = smpool.tile([BPC, D], F32)
        nc.vector.tensor_copy(o_sb[:, 0:512], o_ps[:, 0:512])
        nc.scalar.copy(o_sb[:, 512:D], o_ps[:, 512:D])
        nc.sync.dma_start(out=out_ext[:, 0:512], in_=o_sb[:, 0:512])
        nc.sync.dma_start(out=out_ext[:, 512:D], in_=o_sb[:, 512:D])

    nc.compile()
    return nc


def _prep(desc, nv, Wk, Wv, Wo, attn_w):
    w_k = attn_w[HD:2 * HD]
    w_e = attn_w[2 * HD:]
    Uk = np.einsum('hmd,m->dh', Wk.reshape(H, HD, D), w_k)
    Ue = np.zeros((D, H), np.float32)
    for h in range(H):
        Ue[h * HD:(h + 1) * HD, h] = w_e
    U = np.concatenate([Uk, Ue], axis=1) * USCALE           # [D, 32]
    Up = np.ascontiguousarray(
        U.reshape(DC, 128, 2 * H).swapaxes(0, 1)).astype(NPF8)
    WvTp = np.ascontiguousarray(
        Wv.T.reshape(DC, 128, DC, 128).transpose(1, 2, 0, 3)).astype(NPBF)
    WoTp = np.ascontiguousarray(
        Wo.T.reshape(DC, 128, D).swapaxes(0, 1)).astype(NPBF)
    # nv natural, chunked over j: [B, 128, JC, D]
    nvp = np.ascontiguousarray(
        nv.reshape(B, JC, 128, D).swapaxes(1, 2)).astype(NPBF)
    # nv transposed, chunked over d: [B, 128, DC, S]
    nvTp = np.ascontiguousarray(
        nv.transpose(0, 2, 1).reshape(B, DC, 128, S).swapaxes(1, 2)).astype(NPF8)
    descTp = np.ascontiguousarray(
        desc.transpose(0, 2, 1).reshape(B, DC, 128, S - 1).swapaxes(1, 2)
    ).astype(NPF8)
    return Up, WvTp, WoTp, nvp, nvTp, descTp


def kernel(desc_embeddings, name_value_embeddings, Wq, bq, Wk, bk, Wv, bv,
           attn_w, attn_b, Wo, bo, _trace=False):
    desc = np.asarray(desc_embeddings, np.float32)
    nv = np.asarray(name_value_embeddings, np.float32)
    Up, WvTp, WoTp, nvp, nvTp, descTp = _prep(
        desc, nv, np.asarray(Wk, np.float32), np.asarray(Wv, np.float32),
        np.asarray(Wo, np.float32), np.asarray(attn_w, np.float32))

    if "nc" not in _cache:
        _cache["nc"] = _build()
    nc = _cache["nc"]

    bvr = np.ascontiguousarray(
        np.asarray(bv, np.float32).reshape(DC, 128).T)   # [128, DC]
    in_maps = []
    for c in range(NCORES):
        sl = slice(c * BPC, (c + 1) * BPC)
        in_maps.append({
            "nv": np.ascontiguousarray(nvp[sl]),
            "nvT": np.ascontiguousarray(nvTp[sl]),
            "descT": np.ascontiguousarray(descTp[sl]),
            "U": Up, "WvT": WvTp, "WoT": WoTp, "bv": bvr,
        })
    res = run_bass_kernel_spmd(nc, in_maps, core_ids=list(range(NCORES)),
                               trace=_trace)
    out_rows = np.empty((B, D), np.float32)
    for c in range(NCORES):
        out_rows[c * BPC:(c + 1) * BPC] = res.results[c]["out"]
    out_rows += np.asarray(bo, np.float32)[None, :]
    full = np.broadcast_to(out_rows[:, None, :], (B, S, D))
    if _trace:
        return np.ascontiguousarray(full), res
    return np.ascontiguousarray(full)


# revision 10
# speedup vs baseline: 1.3655x; 1.3655x over previous
"""AdaptiveGraphAttention Trainium2 kernel (8 NeuronCores, data-parallel).

Math: in the reference, logits[b,h,i,j] = a_q[b,h,i] + a_k[b,h,j] +
e_j[b,h,j]*adj[i,j] + attn_b with adj[:,0]=0, adj[:,1:]=1 — the mask and the
j-dependent terms are identical for every query row i, and the a_q/bias terms
are constant over j.  Softmax is shift-invariant, so the attention
distribution p[b,h,:] = softmax_{j>=1}(a_k + e_j) is the same for all i: the
attention matrix is rank-1 and the output is one row per batch, broadcast
over the 256 query positions.  bq/bk/attn_b cancel exactly; bv survives as
an additive constant (sum_j p_j = 1); bo is added on the host.

Per-head dots fold into small matrices:
  a_k[b,j,h] = nv[b,j,:] @ Uk[:,h],  Uk[d,h] = sum_m Wk[h*64+m, d] * w_k[m]
  e_j[b,j,h] = desc[b,j-1,:] @ Ue[:,h], Ue[h*64+m, h] = w_e[m] (else 0)

Device work per core (4 batches), bf16 matmul inputs / f32 accumulation:
  c[h,j]    = Uk.T @ nvT[:, j] + Ue.T @ descT[:, j-1]      (PE, PSUM accum)
  p[h,:]    = softmax_j(c)                                  (DVE/ACT)
  nvbarT    = nv.T @ p.T          [1024, 16] per batch      (PE)
  VbarT     = WvT.T @ nvbarT_all  [1024, 4*16]              (PE)
  ctxT      = blockdiag-select(VbarT) + bv   [1024, 4]      (DVE)
  out       = ctxT.T @ WoT        [4, 1024]                 (PE) -> DMA

All DRAM inputs are host-prepermuted to [128, chunk, inner] so each DMA
partition row is one contiguous run (descriptor-count relief).
"""

import numpy as np
import ml_dtypes
from contextlib import ExitStack

import concourse.bass as bass
import concourse.mybir as mybir
import concourse.tile as tile
from concourse import bacc
from concourse.bass_utils import run_bass_kernel_spmd
from concourse.masks import make_identity

B, S, D, H, HD = 32, 256, 1024, 16, 64
NCORES = 8
BPC = B // NCORES  # 4 batches per core
F32 = mybir.dt.float32
BF16 = mybir.dt.bfloat16
NPBF = ml_dtypes.bfloat16
F8 = mybir.dt.float8e4
NPF8 = ml_dtypes.float8_e4m3
USCALE = 512.0  # fp8 range lift for the tiny folded U entries
DC = D // 128  # 8 chunks of the model dim
JC = S // 128  # 2 chunks of the sequence dim

_cache = {}


def _build():
    nc = bacc.Bacc("TRN2", target_bir_lowering=False, debug=False,
                   num_devices=NCORES)

    nv_ext = nc.declare_dram_parameter("nv", [BPC, 128, JC, D], BF16,
                                       isOutput=False)
    xt_ext = nc.declare_dram_parameter("xT", [BPC, 128, DC, 2 * S - 1], F8,
                                       isOutput=False)
    u_ext = nc.declare_dram_parameter("U", [128, DC, 2 * H], F8,
                                      isOutput=False)
    wvt_ext = nc.declare_dram_parameter("WvT", [128, DC, DC, 128], BF16,
                                        isOutput=False)
    wot_ext = nc.declare_dram_parameter("WoT", [128, DC, D], BF16,
                                        isOutput=False)
    bv_ext = nc.declare_dram_parameter("bv", [128, DC], F32, isOutput=False)
    out_ext = nc.declare_dram_parameter("out", [BPC, D], F32, isOutput=True)

    with tile.TileContext(nc) as tc, ExitStack() as ctx:
        wpool = ctx.enter_context(tc.tile_pool(name="w", bufs=1))
        xpool = ctx.enter_context(tc.tile_pool(name="x", bufs=2))
        smpool = ctx.enter_context(tc.tile_pool(name="sm", bufs=2))
        ps_s = ctx.enter_context(tc.tile_pool(name="ps_s", bufs=2, space="PSUM"))
        ps_pt = ctx.enter_context(tc.tile_pool(name="ps_pt", bufs=1, space="PSUM"))
        ps_nb = ctx.enter_context(tc.tile_pool(name="ps_nb", bufs=2, space="PSUM"))
        ps_vb = ctx.enter_context(tc.tile_pool(name="ps_vb", bufs=1, space="PSUM"))
        ps_o = ctx.enter_context(tc.tile_pool(name="ps_o", bufs=1, space="PSUM"))

        # --- resident constants (weights are DMA'd after the batch loop so
        # the batch-0 activation loads win the DMA queues) ------------------
        ident = wpool.tile([128, 128], F32)
        make_identity(nc, ident[:])
        u_sb = wpool.tile([128, DC, 2 * H], F8)
        nc.sync.dma_start(out=u_sb[:], in_=u_ext.ap())

        # nvbarT for all batches: [p, d-chunk, b*16+h]
        nvall = wpool.tile([128, DC, BPC * H], BF16)

        # weight tiles; chunk DMAs are placed in the queue slack behind the
        # activation loads (activations outrun PE consumption by ~1.5us/batch)
        wvt_sb = wpool.tile([128, DC, DC, 128], BF16)
        wot_sb = wpool.tile([128, DC, D], BF16)
        # weight-chunk DMA schedule per batch-slot: (tile, dram, chunks)
        wsched = {1: [(wvt_sb, wvt_ext, range(0, 4))],
                  2: [(wvt_sb, wvt_ext, range(4, 8)),
                      (wot_sb, wot_ext, range(0, 2))],
                  3: [(wot_sb, wot_ext, range(2, 8))]}

        for b in range(BPC):
            # --- load this batch's activations ----------------------------
            xt_sb = xpool.tile([128, DC, 2 * S - 1], F8)
            nc.sync.dma_start(out=xt_sb[:], in_=xt_ext[b])
            nv_sb = xpool.tile([128, JC, D], BF16)
            nc.sync.dma_start(out=nv_sb[:], in_=nv_ext[b])
            for wtile, wext, chunks in wsched.get(b, []):
                for wc in chunks:
                    nc.sync.dma_start(out=wtile[:, wc], in_=wext[:, wc])

            # --- logits c[h, j] for j=1..255, accumulated in PSUM ---------
            psc = ps_s.tile([H, S - 1], F32)
            for c in range(DC):
                nc.tensor.matmul(psc[:], u_sb[:, c, 0:H],
                                 xt_sb[:, c, 1:S],
                                 start=(c == 0), stop=False)
            for c in range(DC):
                nc.tensor.matmul(psc[:], u_sb[:, c, H:2 * H],
                                 xt_sb[:, c, S:2 * S - 1],
                                 start=False, stop=(c == DC - 1))

            # --- softmax over j (free dim) --------------------------------
            negmax = smpool.tile([H, 1], F32)
            nc.vector.reduce_max(out=negmax[:], in_=psc[:],
                                 axis=mybir.AxisListType.X, negate=True)
            negmax_s = smpool.tile([H, 1], F32)
            nc.vector.tensor_scalar_mul(negmax_s[:], negmax[:], 1.0 / USCALE)
            p_sb = smpool.tile([H, S], F32)
            nc.vector.memset(p_sb[:, 0:1], 0.0)
            sumx = smpool.tile([H, 1], F32)
            nc.scalar.activation(p_sb[:, 1:S], psc[:],
                                 mybir.ActivationFunctionType.Exp,
                                 bias=negmax_s[:], scale=1.0 / USCALE,
                                 accum_out=sumx[:])
            recip = smpool.tile([H, 1], F32)
            nc.vector.reciprocal(recip[:], sumx[:])
            nc.vector.tensor_scalar_mul(p_sb[:, 1:S], p_sb[:, 1:S], recip[:])

            # --- pT[j, h] via PE transpose --------------------------------
            pt_sb = smpool.tile([128, JC, H], BF16)
            for jc in range(JC):
                pt_ps = ps_pt.tile([128, H], F32)
                nc.tensor.transpose(pt_ps[:], p_sb[:, jc * 128:(jc + 1) * 128],
                                    ident[0:H, 0:H])
                nc.vector.tensor_copy(pt_sb[:, jc, :], pt_ps[:])

            # --- nvbarT[d, h] = sum_j nv[j, d] * p[j, h] ------------------
            for cm in range(DC):
                nb_ps = ps_nb.tile([128, H], F32)
                for jc in range(JC):
                    nc.tensor.matmul(nb_ps[:],
                                     nv_sb[:, jc, cm * 128:(cm + 1) * 128],
                                     pt_sb[:, jc, :],
                                     start=(jc == 0), stop=(jc == JC - 1))
                nc.vector.tensor_copy(nvall[:, cm, b * H:(b + 1) * H], nb_ps[:])

        bv_sb = wpool.tile([128, DC], F32)
        nc.sync.dma_start(out=bv_sb[:], in_=bv_ext.ap())

        # --- fused epilogue, pipelined per d'-chunk cm:
        #   VbarT[cm] = sum_ck WvT[ck, cm].T @ nvbarT   [128, 64]
        #   ctxT[cm]  = blockdiag-select + bv           [128, 4]
        #   out      += ctxT[cm].T @ WoT[cm]            [4, 1024] accum
        ctx_sb = wpool.tile([128, DC, BPC], BF16)
        for cm in range(DC):
            vb_ps = ps_vb.tile([128, BPC * H], F32)
            for ck in range(DC):
                nc.tensor.matmul(vb_ps[:],
                                 wvt_sb[:, cm, ck, :],
                                 nvall[:, ck, :],
                                 start=(ck == 0), stop=(ck == DC - 1))
            # block-diagonal select + bv: rows [64*half] take head 2*cm+half
            for half in range(2):
                h = 2 * cm + half
                rows = slice(64 * half, 64 * half + 64)
                src = vb_ps[rows, :].rearrange("p (b h) -> p b h", h=H)[:, :, h]
                nc.vector.tensor_scalar_add(ctx_sb[rows, cm, :], src,
                                            bv_sb[rows, cm:cm + 1])

        # --- out[b, e] = sum_d' ctxT[d', b] * WoT[d', e] ------------------
        o_ps = ps_o.tile([BPC, D], F32)
        for n2 in range(2):
            cols = slice(n2 * 512, (n2 + 1) * 512)
            for ck in range(DC):
                nc.tensor.matmul(o_ps[:, cols], ctx_sb[:, ck, :],
                                 wot_sb[:, ck, cols],
                                 start=(ck == 0), stop=(ck == DC - 1))
        o_sb = smpool.tile([BPC, D], F32)
        nc.vector.tensor_copy(o_sb[:, 0:512], o_ps[:, 0:512])
        nc.scalar.copy(o_sb[:, 512:D], o_ps[:, 512:D])
        nc.sync.dma_start(out=out_ext[:, 0:512], in_=o_sb[:, 0:512])
        nc.sync.dma_start(out=out_ext[:, 512:D], in_=o_sb[:, 512:D])

    nc.compile()
    return nc


def _prep(desc, nv, Wk, Wv, Wo, attn_w):
    w_k = attn_w[HD:2 * HD]
    w_e = attn_w[2 * HD:]
    Uk = np.einsum('hmd,m->dh', Wk.reshape(H, HD, D), w_k)
    Ue = np.zeros((D, H), np.float32)
    for h in range(H):
        Ue[h * HD:(h + 1) * HD, h] = w_e
    U = np.concatenate([Uk, Ue], axis=1) * USCALE           # [D, 32]
    Up = np.ascontiguousarray(
        U.reshape(DC, 128, 2 * H).swapaxes(0, 1)).astype(NPF8)
    WvTp = np.ascontiguousarray(
        Wv.T.reshape(DC, 128, DC, 128).transpose(1, 2, 0, 3)).astype(NPBF)
    WoTp = np.ascontiguousarray(
        Wo.T.reshape(DC, 128, D).swapaxes(0, 1)).astype(NPBF)
    # nv natural, chunked over j: [B, 128, JC, D]
    nvp = np.ascontiguousarray(
        nv.reshape(B, JC, 128, D).swapaxes(1, 2)).astype(NPBF)
    # nv transposed, chunked over d: [B, 128, DC, S]
    nvTp = nv.transpose(0, 2, 1).reshape(B, DC, 128, S).swapaxes(1, 2)
    descTp = desc.transpose(0, 2, 1).reshape(B, DC, 128, S - 1).swapaxes(1, 2)
    xTp = np.concatenate([nvTp, descTp], axis=3).astype(NPF8)
    return Up, WvTp, WoTp, nvp, xTp


def kernel(desc_embeddings, name_value_embeddings, Wq, bq, Wk, bk, Wv, bv,
           attn_w, attn_b, Wo, bo, _trace=False):
    desc = np.asarray(desc_embeddings, np.float32)
    nv = np.asarray(name_value_embeddings, np.float32)
    Up, WvTp, WoTp, nvp, xTp = _prep(
        desc, nv, np.asarray(Wk, np.float32), np.asarray(Wv, np.float32),
        np.asarray(Wo, np.float32), np.asarray(attn_w, np.float32))

    if "nc" not in _cache:
        _cache["nc"] = _build()
    nc = _cache["nc"]

    bvr = np.ascontiguousarray(
        np.asarray(bv, np.float32).reshape(DC, 128).T)   # [128, DC]
    in_maps = []
    for c in range(NCORES):
        sl = slice(c * BPC, (c + 1) * BPC)
        in_maps.append({
            "nv": np.ascontiguousarray(nvp[sl]),
            "xT": np.ascontiguousarray(xTp[sl]),
            "U": Up, "WvT": WvTp, "WoT": WoTp, "bv": bvr,
        })
    res = run_bass_kernel_spmd(nc, in_maps, core_ids=list(range(NCORES)),
                               trace=_trace)
    out_rows = np.empty((B, D), np.float32)
    for c in range(NCORES):
        out_rows[c * BPC:(c + 1) * BPC] = res.results[c]["out"]
    out_rows += np.asarray(bo, np.float32)[None, :]
    full = np.broadcast_to(out_rows[:, None, :], (B, S, D))
    if _trace:
        return np.ascontiguousarray(full), res
    return np.ascontiguousarray(full)
